# revision 3
# baseline (speedup 1.0000x reference)
"""Software-pipelined attention kernel.

Per (head, jt) the PE has 854ns of matmul work (scores + AV) while the
ACT engine needs ~1070ns for the exp, so the baseline's strict
S->exp->AV order stalls the in-order PE queue every step (~1711ns/step).
Here the scores matmuls run TWO jt steps ahead of the AV matmuls
(psS bufs=2 holds exactly two score tiles in flight), the next head
pair's QK-projection matmul chains are interleaved into the remaining
PE slack, and ps_o is drained into SBUF immediately (den+num copies)
so psO (bufs=1) is released before the next head's first AV needs it.
Steady state is PE-bound at ~143us of pure matmul cycles."""

import numpy as np

import concourse.bass as bass
import concourse.mybir as mybir
import concourse.tile as tile
from concourse import bacc
from concourse.alu_op_type import AluOpType
from concourse.bass_utils import run_bass_kernel_spmd

B, N, DIM, H = 8, 1024, 768, 12
DH = DIM // H          # 64
E_QK = 2 * DIM         # 1536
E_V = DIM              # 768
SCALE = DH ** -0.5
NCORES = 8

F32 = mybir.dt.float32
F32R = mybir.dt.float32r
BF16 = mybir.dt.bfloat16

N_TILES = N // 128     # 8
D_CHUNKS = DIM // 128  # 6
QK_TILES = E_QK // 128  # 12
EXP = mybir.ActivationFunctionType.Exp

TIME_REPS = 64


def build_nc(reps=1):
    nc = bacc.Bacc("TRN2", target_bir_lowering=False, debug=False,
                   num_devices=NCORES)

    xT_d = nc.dram_tensor("xT", [DIM, N], BF16, kind="ExternalInput")
    wqk_d = nc.dram_tensor("wqkT", [DIM, E_QK], BF16, kind="ExternalInput")
    wv_d = nc.dram_tensor("wvT", [DIM, E_V], BF16, kind="ExternalInput")
    wp_d = nc.dram_tensor("wpT", [DIM, DIM], BF16, kind="ExternalInput")
    bias_d = nc.dram_tensor("b_proj", [DIM], F32, kind="ExternalInput")
    y_d = nc.dram_tensor("y", [N, DIM], F32, kind="ExternalOutput")

    with tile.TileContext(nc) as tc:
      for _rep in range(reps):
        with tc.tile_pool(name="persist", bufs=1) as persist:
            outT = persist.tile([128, D_CHUNKS, N], BF16)      # 12K/part
            bias_bc = persist.tile([128, DIM], F32)            # 3K/part

            nc.gpsimd.dma_start(
                out=bias_bc[:],
                in_=bias_d.ap()[None, :].broadcast_to([128, DIM]),
            )

            with tc.tile_pool(name="qkv_sb", bufs=1) as qkv_sb:
                qkT = qkv_sb.tile([128, QK_TILES, N], BF16)        # 24K/part
                vp = qkv_sb.tile([128, N_TILES, H, 128], BF16)     # 24K/part

                # wp lives through phase C; allocated below phase A's pools
                with tc.tile_pool(name="cpool", bufs=1) as cpool:
                    wp = cpool.tile([128, D_CHUNKS, DIM], BF16)  # 9K/part

                    with (
                        tc.tile_pool(name="ptpool", bufs=3) as ptpool,
                        tc.tile_pool(name="recpool", bufs=2) as recpool,
                        tc.tile_pool(name="numpool", bufs=1) as numpool,
                        tc.tile_pool(name="psS", bufs=2,
                                     space="PSUM") as psS,
                        tc.tile_pool(name="psO", bufs=1,
                                     space="PSUM") as psO,
                        tc.tile_pool(name="xpool", bufs=1) as xpool,
                        tc.tile_pool(name="wvpool", bufs=1) as wvpool,
                        tc.tile_pool(name="wqkpool", bufs=1) as wqkp,
                        tc.tile_pool(name="psA", bufs=2,
                                     space="PSUM") as psA,
                    ):
                        xT = xpool.tile([128, D_CHUNKS, N], BF16)
                        wv = wvpool.tile([128, D_CHUNKS, E_V], BF16)
                        nc.sync.dma_start(
                            wv[:],
                            wv_d.ap().rearrange("(p dc) f -> p dc f",
                                                dc=D_CHUNKS))
                        xr = xT_d.ap().rearrange(
                            "(p dc) n -> p dc n", dc=D_CHUNKS)
                        nc.sync.dma_start(xT[:, :, 0:256],
                                          xr[:, :, 0:256])
                        nc.sync.dma_start(xT[:, :, 256:N],
                                          xr[:, :, 256:N])
                        wqk = wqkp.tile([128, D_CHUNKS, E_QK], BF16)
                        nc.sync.dma_start(
                            wqk[:],
                            wqk_d.ap().rearrange("(p dc) e -> p dc e",
                                                 dc=D_CHUNKS))
                        nc.sync.dma_start(
                            wp[:], wp_d.ap().rearrange(
                                "(dc p) f -> p dc f", p=128))

                        # memset on f32r fails walrus ISA check; write
                        # the 1.0f pattern through a uint32 view
                        nc.gpsimd.memset(
                            vp[:, :, :, DH:128].bitcast(
                                mybir.dt.uint32),
                            0x3F803F80)

                        # ---- V part ----
                        for jt in range(N_TILES):
                            for fc, fw in ((0, 512), (512, 256)):
                                ps = psA.tile([128, fw], F32,
                                              tag="psA")
                                for dc in range(D_CHUNKS):
                                    nc.tensor.matmul(
                                        ps[:],
                                        xT[:, dc,
                                           jt * 128:(jt + 1) * 128],
                                        wv[:, dc, fc:fc + fw],
                                        start=(dc == 0),
                                        stop=(dc == D_CHUNKS - 1),
                                    )
                                h0, nh = fc // DH, fw // DH
                                nc.vector.tensor_copy(
                                    vp[:, jt, h0:h0 + nh, 0:DH],
                                    ps[:].rearrange(
                                        "p (h c) -> p h c", c=DH),
                                )

                        def qk_chain(et, ncn):
                            ps = psA.tile([128, 512], F32, tag="psA")
                            for dc in range(D_CHUNKS):
                                nc.tensor.matmul(
                                    ps[:],
                                    wqk[:, dc,
                                        et * 128:(et + 1) * 128],
                                    xT[:, dc,
                                       ncn * 512:(ncn + 1) * 512],
                                    start=(dc == 0),
                                    stop=(dc == D_CHUNKS - 1),
                                )
                            nc.vector.tensor_copy(
                                qkT[:, et, ncn * 512:(ncn + 1) * 512],
                                ps[:])

                        def normalize(h, ps_o):
                            # drain ps_o quickly: den+num copies free
                            # psO for the next head's first AV group.
                            # reciprocal_approx_fast needs SBUF input
                            # at partition base 0.
                            base = 64 * (h % 2)
                            den = recpool.tile([64, N], F32, tag="den")
                            nc.vector.tensor_copy(den[0:64, :],
                                                  ps_o[64:128, :])
                            num = numpool.tile([64, N], F32, tag="num")
                            nc.vector.tensor_copy(num[0:64, :],
                                                  ps_o[0:64, :])
                            rec = recpool.tile([64, N], F32, tag="rec")
                            nc.vector.reciprocal_approx_fast(
                                rec[0:64, :], den[0:64, :])
                            nc.vector.tensor_tensor(
                                outT[base:base + 64, h // 2, :],
                                num[0:64, :], rec[0:64, :],
                                op=AluOpType.mult)

                        def attention_head(h, chains):
                            """S matmuls run 2 jt ahead of AV; QK
                            chains for the next pair fill PE slack."""
                            base = 64 * (h % 2)
                            q_et = h // 2
                            k_et = H // 2 + h // 2
                            ps_o = psO.tile([128, N], F32, tag="ps_o")
                            pts = [None] * N_TILES

                            def S(jt):
                                ps_s = psS.tile([128, N], F32,
                                                tag="ps_s")
                                for ic in range(2):
                                    nc.tensor.matmul(
                                        ps_s[:, ic * 512:
                                             (ic + 1) * 512],
                                        qkT[base:base + DH, k_et,
                                            jt * 128:(jt + 1) * 128],
                                        qkT[base:base + DH, q_et,
                                            ic * 512:(ic + 1) * 512],
                                        start=True, stop=True,
                                    )
                                pt = ptpool.tile([128, N], BF16,
                                                 tag="pt")
                                nc.scalar.activation(
                                    pt[:], ps_s[:], EXP, scale=SCALE)
                                pts[jt] = pt

                            def AV(jt):
                                pt = pts[jt]
                                for ic in range(2):
                                    nc.tensor.matmul(
                                        ps_o[:, ic * 512:
                                             (ic + 1) * 512],
                                        vp[:, jt, h, :],
                                        pt[:, ic * 512:(ic + 1) * 512],
                                        start=(jt == 0),
                                        stop=(jt == N_TILES - 1),
                                    )

                            S(0)
                            S(1)
                            if len(chains) > 0:
                                qk_chain(*chains[0])
                            S(2)
                            AV(0)
                            S(3)
                            AV(1)
                            S(4)
                            AV(2)
                            if len(chains) > 1:
                                qk_chain(*chains[1])
                            S(5)
                            AV(3)
                            S(6)
                            AV(4)
                            S(7)
                            AV(5)
                            AV(6)
                            AV(7)
                            normalize(h, ps_o)

                        # QK tiles for pair 0 upfront
                        for et in (0, H // 2):
                            for ncn in range(2):
                                qk_chain(et, ncn)

                        # d loop: attention for pair d with pair d+1's
                        # QK chains interleaved into PE slack
                        for d in range(H // 2):
                            if d + 1 < H // 2:
                                nxt = [(d + 1, 0), (d + 1, 1),
                                       (H // 2 + d + 1, 0),
                                       (H // 2 + d + 1, 1)]
                            else:
                                nxt = []
                            attention_head(2 * d, nxt[:2])
                            attention_head(2 * d + 1, nxt[2:])

                    # ================= Phase C: projection =================
                    with (
                        tc.tile_pool(name="ypool", bufs=2) as ypool,
                        tc.tile_pool(name="psC", bufs=4,
                                     space="PSUM") as psC,
                    ):
                        for nt in range(N_TILES):
                            yt = ypool.tile([128, DIM], F32, tag="yt")
                            for fc, fw in ((0, 512), (512, 256)):
                                ps = psC.tile([128, fw], F32, tag="psC")
                                for dc in range(D_CHUNKS):
                                    nc.tensor.matmul(
                                        ps[:],
                                        outT[:, dc,
                                             nt * 128:(nt + 1) * 128],
                                        wp[:, dc, fc:fc + fw],
                                        start=(dc == 0),
                                        stop=(dc == D_CHUNKS - 1),
                                    )
                                nc.vector.tensor_tensor(
                                    yt[:, fc:fc + fw], ps[:],
                                    bias_bc[:, fc:fc + fw],
                                    op=AluOpType.add)
                            nc.sync.dma_start(
                                y_d.ap().rearrange("(nt p) f -> p nt f",
                                                   p=128)[:, nt, :],
                                yt[:])

    nc.compile()
    return nc


def make_runner(nc, mesh):
    """jit(shard_map(bass_exec)) runner over 8 cores, no donation."""
    import jax
    from jax.experimental.shard_map import shard_map
    from jax.sharding import PartitionSpec
    from concourse import bass2jax, mybir as _mb

    bass2jax.install_neuronx_cc_hook()

    partition_name = (nc.partition_id_tensor.name
                      if nc.partition_id_tensor else None)
    in_names, out_names, out_avals, zero_outs = [], [], [], []
    for alloc in nc.m.functions[0].allocations:
        if not isinstance(alloc, _mb.MemoryLocationSet):
            continue
        name = alloc.memorylocations[0].name
        if alloc.kind == "ExternalInput":
            if name != partition_name:
                in_names.append(name)
        elif alloc.kind == "ExternalOutput":
            out_names.append(name)
            out_avals.append(jax.core.ShapedArray(
                tuple(alloc.tensor_shape), _mb.dt.np(alloc.dtype)))
            zero_outs.append(np.zeros(
                tuple(alloc.tensor_shape), _mb.dt.np(alloc.dtype)))

    all_in_names = list(in_names) + list(out_names)
    if partition_name is not None:
        all_in_names = all_in_names + [partition_name]

    def _body(*args):
        operands = list(args)
        if partition_name is not None:
            operands.append(bass2jax.partition_id_tensor())
        outs = bass2jax._bass_exec_p.bind(
            *operands,
            out_avals=tuple(out_avals),
            in_names=tuple(all_in_names),
            out_names=tuple(out_names),
            lowering_input_output_aliases=(),
            sim_require_finite=True,
            sim_require_nnan=True,
            nc=nc,
        )
        return tuple(outs)

    n_params = len(in_names)
    n_outs = len(out_names)
    specs = (PartitionSpec("core"),) * (n_params + n_outs)
    jitted = jax.jit(
        shard_map(_body, mesh=mesh, in_specs=specs,
                  out_specs=(PartitionSpec("core"),) * n_outs,
                  check_rep=False),
        keep_unused=True,
    )
    return jitted, in_names, out_names, zero_outs


def _prep_inputs(x, w_qkv, w_proj, b_proj):
    import ml_dtypes
    x = np.ascontiguousarray(np.asarray(x, dtype=np.float32))
    w_qkv = np.asarray(w_qkv, dtype=np.float32)
    w_proj = np.asarray(w_proj, dtype=np.float32)
    b_proj = np.ascontiguousarray(np.asarray(b_proj, dtype=np.float32))

    bf16 = ml_dtypes.bfloat16
    xT = np.ascontiguousarray(x.transpose(0, 2, 1)).astype(bf16)  # [B, D, N]
    wqkT = np.ascontiguousarray(w_qkv[:E_QK].T).astype(bf16)      # [D, 2D]
    wvT = np.ascontiguousarray(w_qkv[E_QK:].T).astype(bf16)       # [D, D]
    wpT = np.ascontiguousarray(w_proj.T).astype(bf16)             # [D, D]
    per_core = {"xT": None, "wqkT": wqkT, "wvT": wvT, "wpT": wpT,
                "b_proj": b_proj}

    def core_map(b):
        m = dict(per_core)
        m["xT"] = xT[b]
        return m

    return [core_map(b) for b in range(NCORES)]


_NC_CACHE = None


def _get_nc():
    global _NC_CACHE
    if _NC_CACHE is None:
        _NC_CACHE = build_nc()
    return _NC_CACHE


def kernel(x, w_qkv, w_proj, b_proj):
    in_maps = _prep_inputs(x, w_qkv, w_proj, b_proj)
    res = run_bass_kernel_spmd(_get_nc(), in_maps,
                               core_ids=list(range(NCORES)))
    return np.stack([res.results[b]["y"] for b in range(NCORES)], axis=0)
